# revision 24
# baseline (speedup 1.0000x reference)
"""Cosine-similarity kernel (x[16384,512] vs weights[4096,512] -> [16384,4096])
on 8 Trainium2 NeuronCores, data-parallel over the x batch dim.

Per core: x shard [2048,512] fp32, full weights [4096,512] fp32.
  out = normalize(x) @ normalize(w).T

Implementation: rows are normalized and scaled by S=32 (square+reduce on
ACT/DVE, DVE reciprocal, ACT sqrt, Pool scalar-mul -> bf16), PE-transposed
(bf16, 1 cyc/row), then split into fp8e4m3 hi+lo parts (hi = fp8(v),
lo = fp8(v - hi)). The GEMM runs as 3-term fp8 DoubleRow matmuls
(hi*hi + hi*lo + lo*hi), each contracting 2 k-tiles per instruction
(lhsT [128,2,128], rhs [128,2,512] -> 0.5 cyc/row), accumulating all 6
instructions per (m, nb) into fp32 PSUM. PSUM pairs are evicted with a
1/S^2 scale to fp16 and DMA'd out; the host upcasts to fp32.

Schedule: all input DMAs are issued up front (the DMA wire is a serial
~360GB/s resource, so inputs get priority); the first two n-blocks are
computed as single-bank groups in data-arrival order with their output
DMAs deferred until the inputs finish; the remaining three blocks run as
2-bank pairs with next-block w prep staged (Pool mul early, PE transpose
later) inside the matmul stream. Inputs are host-permuted to
partition-major [128, T, 512] so each input DMA moves 4 row-tiles with
2KB-contiguous descriptors.
"""
import numpy as np

B, D, N = 16384, 512, 4096
SQA = 4   # of 6 row-norm squares -> ACT (rest DVE)
HIA = 4   # of 4 hi-quantize ops -> ACT (rest DVE)
EVA = 4   # of 8 PSUM evictions -> ACT (rest DVE)
NCORES = 8
BS = B // NCORES          # 2048 rows per core
MT = BS // 128            # 16 x tiles
JT = N // 128             # 32 w tiles
S = 32.0                  # fp8 quantization pre-scale

_cached = {}


def _build():
    import concourse.bass as bass
    import concourse.mybir as mybir
    import concourse.tile as tile
    from concourse import bacc
    from concourse.masks import make_identity

    F32 = mybir.dt.float32
    F8 = mybir.dt.float8e4
    BF16 = mybir.dt.bfloat16
    F16 = mybir.dt.float16
    DR = mybir.MatmulPerfMode.DoubleRow
    AF = mybir.ActivationFunctionType

    nc = bacc.Bacc(None, target_bir_lowering=False)
    xp = nc.dram_tensor("x", [128, MT, D], F32, kind="ExternalInput")
    wp = nc.dram_tensor("weights", [128, JT, D], F32, kind="ExternalInput")
    o = nc.dram_tensor("out", [BS, N], F16, kind="ExternalOutput")

    with tile.TileContext(nc) as tc:
        with (
            tc.tile_pool(name="const", bufs=1) as const,
            tc.tile_pool(name="big", bufs=1) as big,
            tc.tile_pool(name="ld", bufs=7) as ldp,
            tc.tile_pool(name="ldx", bufs=1) as ldxp,
            tc.tile_pool(name="st", bufs=3) as stp,
            tc.tile_pool(name="nbp", bufs=6) as nbp_pool,
            tc.tile_pool(name="ot", bufs=5) as otp,
            tc.tile_pool(name="od", bufs=33) as odp,
            tc.tile_pool(name="ptps", bufs=2, space="PSUM") as ptps,
            tc.tile_pool(name="mmps", bufs=3, space="PSUM") as mmps,
        ):
            ident = const.tile([128, 128], BF16, name="ident")
            make_identity(nc, ident[:])

            # Preload activation tables (Square/Sqrt/Copy) during DMA latency.
            dum = const.tile([128, 1], F32, name="dum")
            nc.vector.memset(dum[:], 1.0)
            d2 = const.tile([128, 1], F32, name="d2")
            d3 = const.tile([128, 1], F32, name="d3")
            nc.scalar.activation(d2[:], dum[:], AF.Square, accum_out=d3[:])
            nc.scalar.activation(d2[:], dum[:], AF.Sqrt, scale=1.0)
            nc.scalar.copy(d2[:], dum[:])
            nc.scalar.mul(d2[:], dum[:], 1.0)

            xh = big.tile([128, 4, BS], F8, name="xh")
            xl = big.tile([128, 4, BS], F8, name="xl")
            wh = big.tile([128, 4, N], F8, name="wh")
            wl = big.tile([128, 4, N], F8, name="wl")

            state = {"sq": 0, "ev": 0}

            def load_chunk(src, c0, cn, tag="ld"):
                pool = ldp if tag == "ld" else ldxp
                ch = pool.tile([128, cn, D], F32, name=f"ld{c0}", tag=tag)
                nc.sync.dma_start(ch[:], src[:, c0:c0 + cn, :])
                return ch

            def norm_tile(t):
                """-> rs[128,1] = S / ||row||, engines rotated for squares."""
                ss = stp.tile([128, 1], F32, name="ss", tag="ss")
                n = state["sq"]
                state["sq"] += 1
                if (n * SQA) % 6 < SQA:
                    sq = stp.tile([128, D], F32, name="sq", tag="sqa")
                    nc.scalar.activation(sq[:], t, AF.Square, accum_out=ss[:])
                else:
                    sq = stp.tile([128, D], F32, name="sqd", tag="sqd")
                    nc.vector.tensor_tensor(sq[:], t, t, mybir.AluOpType.mult)
                    nc.vector.tensor_reduce(ss[:], sq[:], mybir.AxisListType.X,
                                            mybir.AluOpType.add)
                inv = stp.tile([128, 1], F32, name="inv", tag="inv")
                nc.vector.reciprocal(inv[:], ss[:])
                rs = stp.tile([128, 1], F32, name="rs", tag="rs")
                nc.scalar.activation(rs[:], inv[:], AF.Sqrt, scale=float(S * S))
                return rs

            def prep_mul(ch, i0, un):
                """Normalize+scale `un` tiles -> list of bf16 nb tiles."""
                nbs = []
                for u in range(un):
                    t = ch[:, i0 + u, :]
                    rs = norm_tile(t)
                    nb = nbp_pool.tile([128, D], BF16, name="nb", tag="nb")
                    nc.gpsimd.tensor_scalar_mul(nb[:], t, rs[:])
                    nbs.append(nb)
                return nbs

            def prep_tr(nbs, hi, lo, col):
                """Transpose bf16 tiles into PSUM, fp8 hi/lo split."""
                un = len(nbs)
                pt = ptps.tile([128, 4, 2, 128], BF16, name="pt", tag="pt")
                for u, nb in enumerate(nbs):
                    for k in range(4):
                        nc.tensor.transpose(pt[:, k, u, :],
                                            nb[:, k * 128:(k + 1) * 128], ident[:])
                w = 128 * un
                state["hi"] = state.get("hi", 0) + 1
                if (state["hi"] * HIA) % 4 < HIA:
                    nc.scalar.copy(hi[:, :, col:col + w], pt[:, :, 0:un, :])
                else:
                    nc.vector.tensor_copy(hi[:, :, col:col + w],
                                          pt[:, :, 0:un, :])
                nc.vector.tensor_tensor(lo[:, :, col:col + w], pt[:, :, 0:un, :],
                                        hi[:, :, col:col + w],
                                        mybir.AluOpType.subtract)

            def prepn(ch, i0, un, hi, lo, col):
                prep_tr(prep_mul(ch, i0, un), hi, lo, col)

            def mm_terms(pm_slice, m, nbi):
                idx = 0
                for (a, b) in ((xh, wh), (xh, wl), (xl, wh)):
                    for kk in (0, 1):
                        nc.tensor.matmul(
                            pm_slice,
                            a[:, 2 * kk:2 * kk + 2, m * 128:(m + 1) * 128],
                            b[:, 2 * kk:2 * kk + 2, nbi * 512:(nbi + 1) * 512],
                            start=(idx == 0), stop=(idx == 5), perf_mode=DR)
                        idx += 1

            def evict(ot_ap, pm_ap, eng=None):
                ev = state["ev"]
                state["ev"] += 1
                if eng == "act" or (eng is None and (ev * EVA) % 8 < EVA):
                    nc.scalar.mul(ot_ap, pm_ap, float(1.0 / (S * S)))
                else:
                    nc.vector.tensor_scalar_mul(ot_ap, pm_ap, float(1.0 / (S * S)))

            deferred = []

            def mm_single(m, nbi, defer=False):
                """6 DoubleRow matmuls -> 1-bank PSUM; evict+store [128,512]."""
                pm = mmps.tile([128, 2, D], F32, name="pms", tag="pm")
                mm_terms(pm[:, 0, :], m, nbi)
                pool = odp
                ot = pool.tile([128, D], F16, name="ot1", tag="ot1")
                evict(ot[:], pm[:, 0, :])
                if defer:
                    deferred.append((m, nbi, ot))
                else:
                    nc.sync.dma_start(
                        o[m * 128:(m + 1) * 128, nbi * 512:(nbi + 1) * 512], ot[:])

            def flush_outs():
                for m, nbi, ot in deferred:
                    nc.sync.dma_start(
                        o[m * 128:(m + 1) * 128, nbi * 512:(nbi + 1) * 512], ot[:])
                deferred.clear()

            def mm_pair(m, nbp):
                """6 DoubleRow matmuls per nb x 2 nbs -> 2-bank PSUM; evict+store."""
                pm = mmps.tile([128, 2, D], F32, name="pm", tag="pm")
                for i in (0, 1):
                    mm_terms(pm[:, i, :], m, 2 * nbp + i)
                ot = otp.tile([128, 2, D], F16, name="ot", tag="ot")
                evict(ot[:], pm[:])
                nc.sync.dma_start(
                    o[m * 128:(m + 1) * 128, nbp * 1024:(nbp + 1) * 1024], ot[:])

            # ---- startup: all input loads issued up front (serial DMA wire
            # stays input-only until the last w block lands); nb0/nb1 computed
            # as singles in data-arrival order with output DMAs deferred ----
            x0 = load_chunk(xp, 0, 1, tag="ld1")
            w0a = load_chunk(wp, 0, 4)
            x0b = load_chunk(xp, 1, 3, tag="ld3")
            xw1 = load_chunk(xp, 4, 4)
            w0b = load_chunk(wp, 4, 4)
            xw2 = load_chunk(xp, 8, 4)
            xw3 = load_chunk(xp, 12, 4)
            w1a = load_chunk(wp, 8, 4)
            w1b = load_chunk(wp, 12, 4)
            prepn(x0, 0, 1, xh, xl, 0)
            prepn(w0a, 0, 2, wh, wl, 0)
            prepn(w0a, 2, 2, wh, wl, 256)
            mm_single(0, 0, defer=True)
            prepn(x0b, 0, 1, xh, xl, 128)
            mm_single(1, 0, defer=True)
            prepn(x0b, 1, 2, xh, xl, 256)
            mm_single(2, 0, defer=True)
            mm_single(3, 0, defer=True)
            prepn(xw1, 0, 2, xh, xl, 512)
            mm_single(4, 0, defer=True)
            prepn(xw1, 2, 2, xh, xl, 768)
            mm_single(5, 0, defer=True)
            prepn(w0b, 0, 2, wh, wl, 512)
            mm_single(6, 0, defer=True)
            prepn(w0b, 2, 2, wh, wl, 768)
            mm_single(7, 0, defer=True)
            for m in range(0, 4):
                mm_single(m, 1, defer=True)
            prepn(xw2, 0, 2, xh, xl, 1024)
            for m in range(4, 8):
                mm_single(m, 1, defer=True)
            prepn(xw2, 2, 2, xh, xl, 1280)
            mm_single(8, 0, defer=True)
            mm_single(9, 0, defer=True)
            prepn(xw3, 0, 2, xh, xl, 1536)
            mm_single(10, 0, defer=True)
            mm_single(11, 0, defer=True)
            prepn(xw3, 2, 2, xh, xl, 1792)
            mm_single(12, 0, defer=True)
            mm_single(13, 0, defer=True)
            Nm = prep_mul(w1a, 0, 2)
            mm_single(14, 0, defer=True)
            mm_single(15, 0, defer=True)
            Om = prep_mul(w1a, 2, 2)
            mm_single(8, 1, defer=True)
            mm_single(9, 1, defer=True)
            Pm = prep_mul(w1b, 0, 2)
            mm_single(10, 1, defer=True)
            mm_single(11, 1, defer=True)
            Qm = prep_mul(w1b, 2, 2)
            w2a = load_chunk(wp, 16, 4)
            w2b = load_chunk(wp, 20, 4)
            for m in range(12, MT):
                mm_single(m, 1, defer=True)
            prep_tr(Nm, wh, wl, 1024)
            prep_tr(Om, wh, wl, 1280)
            prep_tr(Pm, wh, wl, 1536)
            prep_tr(Qm, wh, wl, 1792)
            flush_outs()

            # ---- remaining nb-pair blocks; w chunks prefetched a block
            # ahead, prepped early in each block ----
            wcur = (w2a, w2b, 16)
            w3 = [None, None]
            for nbp in range(1, 4):
                for m in range(MT):
                    if nbp == 3 and m == MT - 1:
                        break
                    mm_pair(m, nbp)
                    if wcur is not None:
                        if m == 0:
                            pma = prep_mul(wcur[0], 0, 2)
                        elif m == 1:
                            pmb = prep_mul(wcur[0], 2, 2)
                        elif m == 2:
                            prep_tr(pma, wh, wl, wcur[2] * 128)
                        elif m == 3:
                            prep_tr(pmb, wh, wl, wcur[2] * 128 + 256)
                        elif m == 4:
                            pmc = prep_mul(wcur[1], 0, 2)
                        elif m == 5:
                            pmd = prep_mul(wcur[1], 2, 2)
                        elif m == 6:
                            prep_tr(pmc, wh, wl, (wcur[2] + 4) * 128)
                        elif m == 7:
                            prep_tr(pmd, wh, wl, (wcur[2] + 4) * 128 + 256)
                    if nbp == 1 and m == 9:
                        w3[0] = load_chunk(wp, 24, 4)
                    elif nbp == 1 and m == 11:
                        w3[1] = load_chunk(wp, 28, 4)
                wcur = (w3[0], w3[1], 24) if nbp == 1 else None
            # tail: last output tile as two singles (smaller final evict+DMA)
            mm_single(MT - 1, 6)
            mm_single(MT - 1, 7)
    nc.compile()
    return nc


def kernel(x: np.ndarray, weights: np.ndarray) -> np.ndarray:
    from concourse.bass_utils import run_bass_kernel_spmd

    if "nc" not in _cached:
        _cached["nc"] = _build()
    nc = _cached["nc"]

    x = np.ascontiguousarray(x, dtype=np.float32)
    weights = np.ascontiguousarray(weights, dtype=np.float32)
    # partition-major layouts: [128, tiles, D]
    xs = x.reshape(NCORES, MT, 128, D)
    wperm = np.ascontiguousarray(
        weights.reshape(JT, 128, D).transpose(1, 0, 2))
    in_maps = [
        {"x": np.ascontiguousarray(xs[i].transpose(1, 0, 2)), "weights": wperm}
        for i in range(NCORES)
    ]
    res = run_bass_kernel_spmd(nc, in_maps, list(range(NCORES)))
    out = np.concatenate([res.results[i]["out"] for i in range(NCORES)], axis=0)
    return out.astype(np.float32)

